# revision 15
# baseline (speedup 1.0000x reference)
"""TRN2 Bass kernel: 2-bit-quantized linear  y = x @ (levels[idx] * scale).T + bias.

Sharding: column-parallel over 8 NeuronCores - each core owns OUT_F/8 output
features (its slice of the weights / scales / bias); x is replicated.

The kernel is PE-streaming-bound: per core 12 o-tiles x 32 k-tiles x 8
token-chunks = 3072 matmuls of [128k,128o]x[128k,512t], whose issue floor is
512 cycles @ 2.4 GHz + ~2.5 ns NX overhead = ~663 us.  Everything else is
engineered to keep the PE at that floor:

  - Host-side dequant: W^T[k,o] = e3m4(levels/s)[idx^T].  fp8e3 (E3M4, 5
    significant bits) weights stream at full PE rate, halve W DMA traffic
    and SBUF footprint vs fp16, and with the best global pre-scale s (folded
    into the drain scale) cost only ~0.3-0.8% relative error - far inside
    the 2e-2 budget.  (fp8e4 DoubleRow 2x matmul was analyzed and rejected:
    e4m3 x-quantization alone is ~2.7% error, over budget, and any 2-digit
    compensation scheme doubles the matmul count, cancelling the 2x rate.)
  - x^T fp16 is streamed in [128, 512]-token chunks (PSUM-bank-limited
    moving size), prefetched 2 chunks ahead.
  - Startup-critical bytes are spread over all three HW DGE queues (first
    W/x slices on gpsimd, W on sync, chunk-0 x + output on scalar), with
    tiny bsv/scl loads at the sync/scalar queue heads to absorb the
    cold-DMA-path latency.
  - 32 warmup matmuls (N=128, on a zeroed tile) bridge the DMA startup
    window so the HAM clock gate reaches 2.4 GHz as the real stream begins.
  - Chunk 0 runs k-outer across 8 PSUM banks (8 o-tiles accumulate in
    lockstep) so the PE consumes each W k-tile the moment its DMA lands;
    remaining o-tiles and later chunks run k-inner with all weights resident.
  - PSUM drain fuses scale and bias via one ScalarE activation with
    per-partition scale/bias vectors, writing y^T fp16 (halves output DMA);
    the final o-group is split into progressively smaller token groups to
    shorten the drain tail.

The host transposes x / W on the way in and y^T on the way out; those are
layout moves only.
"""

import numpy as np
import ml_dtypes

import concourse.bass as bass
import concourse.bacc as bacc
import concourse.tile as tile
import concourse.mybir as mybir
from concourse.bass_utils import run_bass_kernel_spmd

AF = mybir.ActivationFunctionType
DT = mybir.dt

NCORES = 8

# Problem sizes (hardcoded per contract).
B, S, IN_F, OUT_F = 4, 1024, 4096, 12288
T_TOKENS = B * S
O_SHARD = OUT_F // NCORES


def build_program(
    *,
    in_f: int,
    t_tokens: int,
    o_shard: int,
    tc_size: int = 512,
    x_extra_bufs: int = 32,
    out_bufs: int = 6,
    ramp_banks: int = 8,
    warmup_mms: int = 32,
):
    """Build the single-core Bass/Tile program (SPMD across cores)."""
    assert in_f % 128 == 0 and o_shard % 128 == 0 and t_tokens % tc_size == 0
    kt = in_f // 128
    n_ot = o_shard // 128
    n_tc = t_tokens // tc_size

    nc = bacc.Bacc("TRN2", target_bir_lowering=False, debug=False)

    xt_d = nc.dram_tensor("xt", [in_f, t_tokens], DT.float16, kind="ExternalInput")
    wt_d = nc.dram_tensor("wt", [in_f, o_shard], DT.float8e3, kind="ExternalInput")
    scl_d = nc.dram_tensor("scl", [128, n_ot], DT.float32, kind="ExternalInput")
    bsv_d = nc.dram_tensor("bsv", [128, n_ot], DT.float32, kind="ExternalInput")
    yt_d = nc.dram_tensor("yt", [o_shard, t_tokens], DT.float16, kind="ExternalOutput")

    with tile.TileContext(nc) as tc:
        with (
            tc.tile_pool(name="const", bufs=1) as cpool,
            tc.tile_pool(name="wt", bufs=kt) as wtp,
            tc.tile_pool(name="xtp", bufs=kt + x_extra_bufs) as xtp,
            tc.tile_pool(name="outp", bufs=out_bufs) as outp,
            tc.tile_pool(name="ps", bufs=8, space=bass.MemorySpace.PSUM) as psp,
        ):
            # PE warmup: a few dummy matmuls on a zeroed tile start the HAM
            # activity window while the first DMAs land, so the real matmul
            # stream reaches the warm 2.4 GHz clock sooner.
            if warmup_mms:
                warm_t = cpool.tile([128, 128], DT.float16, tag="warm")
                nc.gpsimd.memset(warm_t[:], 0.0)
                warm_ps = psp.tile([128, 128], DT.float32, tag="ps", name="warm")
                for _ in range(warmup_mms):
                    nc.tensor.matmul(
                        warm_ps[:], warm_t[:], warm_t[:], start=True, stop=True
                    )

            # Tiny constant loads go first on each HW DGE queue: they warm
            # the DMA path so the first W / x transfers run at full rate.
            bsv_t = cpool.tile([128, n_ot], DT.float32, tag="bsv")
            nc.sync.dma_start(bsv_t[:], bsv_d[:])
            scl_t = cpool.tile([128, n_ot], DT.float32, tag="scl")
            nc.scalar.dma_start(scl_t[:], scl_d[:])

            # W k-tile loads on the sync queue; chunk-0 x goes on the scalar
            # queue so the two transfer concurrently during the ramp.
            wts = []
            w0_split = min(3, n_ot) * 128
            for k in range(kt):
                w = wtp.tile([128, o_shard], DT.float8e3, tag="wt")
                if k == 0 and w0_split < o_shard:
                    # First k-tile split across two queues: the ramp's first
                    # matmuls need only the leading o-columns, which land on
                    # the otherwise-idle gpsimd queue right after the memset.
                    nc.gpsimd.dma_start(w[:, 0:w0_split], wt_d[0:128, 0:w0_split])
                    nc.sync.dma_start(
                        w[:, w0_split:o_shard], wt_d[0:128, w0_split:o_shard]
                    )
                else:
                    nc.sync.dma_start(w[:], wt_d[k * 128 : (k + 1) * 128, :])
                wts.append(w)

            def load_chunk(tci):
                xts = []
                for k in range(kt):
                    if tci == 0:
                        eng = nc.gpsimd if k == 0 else nc.scalar
                    else:
                        eng = nc.sync
                    xt_t = xtp.tile([128, tc_size], DT.float16, tag="xt")
                    eng.dma_start(
                        xt_t[:],
                        xt_d[
                            k * 128 : (k + 1) * 128,
                            tci * tc_size : (tci + 1) * tc_size,
                        ],
                    )
                    xts.append(xt_t)
                return xts

            def drain_store(ps, ot, tci, t0=0, tn=None):
                tn = tc_size if tn is None else tn
                out_t = outp.tile([128, tn], DT.float16, tag="out")
                nc.scalar.activation(
                    out_t[:],
                    ps[:],
                    AF.Identity,
                    bias=bsv_t[:, ot : ot + 1],
                    scale=scl_t[:, ot : ot + 1],
                )
                nc.scalar.dma_start(
                    yt_d[
                        ot * 128 : (ot + 1) * 128,
                        tci * tc_size + t0 : tci * tc_size + t0 + tn,
                    ],
                    out_t[:],
                )

            def mm_group(xts, ot, tci, t0=0, tn=None):
                tn = tc_size if tn is None else tn
                ps = psp.tile([128, tn], DT.float32, tag="ps")
                for k in range(kt):
                    nc.tensor.matmul(
                        ps[:],
                        wts[k][:, ot * 128 : (ot + 1) * 128],
                        xts[k][:, t0 : t0 + tn],
                        start=(k == 0),
                        stop=(k == kt - 1),
                    )
                drain_store(ps, ot, tci, t0, tn)

            for tci in range(n_tc):
                xts = load_chunk(tci)
                if tci == 0 and ramp_banks:
                    # k-outer across `ramp_banks` PSUM banks: the PE consumes
                    # each (W, x) k-tile pair the moment the DMA delivers it.
                    ra = list(range(min(ramp_banks, n_ot)))
                    pss = {
                        ot: psp.tile([128, tc_size], DT.float32, tag="ps", name="ps")
                        for ot in ra
                    }
                    for k in range(kt):
                        for ot in ra:
                            nc.tensor.matmul(
                                pss[ot][:],
                                wts[k][:, ot * 128 : (ot + 1) * 128],
                                xts[k][:],
                                start=(k == 0),
                                stop=(k == kt - 1),
                            )
                    for ot in ra:
                        drain_store(pss[ot], ot, tci)
                    rest = range(len(ra), n_ot)
                else:
                    rest = range(n_ot)
                for ot in rest:
                    if tci == n_tc - 1 and ot == n_ot - 1 and tc_size >= 512:
                        h = tc_size // 2
                        q = tc_size // 4
                        mm_group(xts, ot, tci, 0, h)
                        mm_group(xts, ot, tci, h, q)
                        mm_group(xts, ot, tci, h + q, q)
                    else:
                        mm_group(xts, ot, tci)

    nc.compile()
    return nc


def quant_levels(levels):
    """Quantize the 4 levels to fp8 e3m4 with the best global pre-scale s
    (folded back into the per-output drain scale)."""
    lv = np.asarray(levels, dtype=np.float64)
    best = (np.inf, 1.0)
    for sc in np.geomspace(0.25, 4.0, 257):
        q = np.asarray(lv / sc, dtype=np.float32).astype(
            ml_dtypes.float8_e3m4).astype(np.float64) * sc
        r = float(np.sqrt(np.mean((q - lv) ** 2)))
        if r < best[0]:
            best = (r, float(sc))
    sc = best[1]
    lut = np.asarray(lv / sc, dtype=np.float32).astype(ml_dtypes.float8_e3m4)
    return lut, sc


def make_in_maps(x, levels, weight_indices, weight_scales, bias):
    """Host-side shard + layout prep: one input map per core."""
    t_tokens = x.shape[0] * x.shape[1]
    in_f = x.shape[2]
    o_shard = weight_indices.shape[0] // NCORES
    n_ot = o_shard // 128

    x2 = np.asarray(x, dtype=np.float32).reshape(t_tokens, in_f)
    xt = np.ascontiguousarray(x2.T).astype(np.float16)

    # Host dequant: W^T[k, o] = e3m4(levels/s)[idx^T[k, o]]; s and the
    # per-output scale are folded into the drain.
    lut, lv_scale = quant_levels(levels)
    wt_full = lut[np.asarray(weight_indices).T]  # [IN_F, OUT_F] fp8e3

    in_maps = []
    for c in range(NCORES):
        o0, o1 = c * o_shard, (c + 1) * o_shard
        wt = np.ascontiguousarray(wt_full[:, o0:o1])
        scl = np.ascontiguousarray(
            (np.asarray(weight_scales[o0:o1], dtype=np.float64) * lv_scale)
            .astype(np.float32).reshape(n_ot, 128).T
        )
        bsv = np.ascontiguousarray(
            np.asarray(bias[o0:o1], dtype=np.float32).reshape(n_ot, 128).T
        )
        in_maps.append({"xt": xt, "wt": wt, "scl": scl, "bsv": bsv})
    return in_maps


_PROGRAM_CACHE: dict = {}


def _get_program():
    if "p" not in _PROGRAM_CACHE:
        _PROGRAM_CACHE["p"] = build_program(
            in_f=IN_F, t_tokens=T_TOKENS, o_shard=O_SHARD
        )
    return _PROGRAM_CACHE["p"]


def run_on_cores(x, levels, weight_indices, weight_scales, bias, *, trace=False):
    nc = _get_program()
    in_maps = make_in_maps(x, levels, weight_indices, weight_scales, bias)
    res = run_bass_kernel_spmd(
        nc, in_maps, core_ids=list(range(NCORES)), trace=trace
    )
    yt = np.concatenate([res.results[c]["yt"] for c in range(NCORES)], axis=0)
    y = np.ascontiguousarray(yt.T).astype(np.float32).reshape(B, S, OUT_F)
    return y, res


def kernel(x, levels, weight_indices, weight_scales, bias):
    y, _ = run_on_cores(x, levels, weight_indices, weight_scales, bias)
    return y


# revision 16
# speedup vs baseline: 1.0011x; 1.0011x over previous
"""TRN2 Bass kernel: 2-bit-quantized linear  y = x @ (levels[idx] * scale).T + bias.

Sharding: column-parallel over 8 NeuronCores - each core owns OUT_F/8 output
features (its slice of the weights / scales / bias); x is replicated.

The kernel is PE-streaming-bound: per core 12 o-tiles x 32 k-tiles x 8
token-chunks = 3072 matmuls of [128k,128o]x[128k,512t], whose issue floor is
512 cycles @ 2.4 GHz + ~2.5 ns NX overhead = ~663 us.  Everything else is
engineered to keep the PE at that floor:

  - Host-side dequant: W^T[k,o] = e3m4(levels/s)[idx^T].  fp8e3 (E3M4, 5
    significant bits) weights stream at full PE rate, halve W DMA traffic
    and SBUF footprint vs fp16, and with the best global pre-scale s (folded
    into the drain scale) cost only ~0.3-0.8% relative error - far inside
    the 2e-2 budget.  (fp8e4 DoubleRow 2x matmul was analyzed and rejected:
    e4m3 x-quantization alone is ~2.7% error, over budget, and any 2-digit
    compensation scheme doubles the matmul count, cancelling the 2x rate.)
  - x^T fp16 is streamed in [128, 512]-token chunks (PSUM-bank-limited
    moving size), prefetched 2 chunks ahead.
  - Startup-critical bytes are spread over all three HW DGE queues (first
    W/x slices on gpsimd, W on sync, chunk-0 x + output on scalar), with
    tiny bsv/scl loads at the sync/scalar queue heads to absorb the
    cold-DMA-path latency.
  - 32 warmup matmuls (N=128, on a zeroed tile) bridge the DMA startup
    window so the HAM clock gate reaches 2.4 GHz as the real stream begins.
  - Chunk 0 runs k-outer across 8 PSUM banks (8 o-tiles accumulate in
    lockstep) so the PE consumes each W k-tile the moment its DMA lands;
    remaining o-tiles and later chunks run k-inner with all weights resident.
  - PSUM drain fuses scale and bias via one ScalarE activation with
    per-partition scale/bias vectors, writing y^T fp16 (halves output DMA);
    the final o-group is split into progressively smaller token groups to
    shorten the drain tail.

The host transposes x / W on the way in and y^T on the way out; those are
layout moves only.
"""

import numpy as np
import ml_dtypes

import concourse.bass as bass
import concourse.bacc as bacc
import concourse.tile as tile
import concourse.mybir as mybir
from concourse.bass_utils import run_bass_kernel_spmd

AF = mybir.ActivationFunctionType
ALU = mybir.AluOpType
DT = mybir.dt

NCORES = 8

# Problem sizes (hardcoded per contract).
B, S, IN_F, OUT_F = 4, 1024, 4096, 12288
T_TOKENS = B * S
O_SHARD = OUT_F // NCORES


def build_program(
    *,
    in_f: int,
    t_tokens: int,
    o_shard: int,
    tc_size: int = 512,
    x_extra_bufs: int = 32,
    out_bufs: int = 6,
    ramp_banks: int = 8,
    warmup_mms: int = 32,
):
    """Build the single-core Bass/Tile program (SPMD across cores)."""
    assert in_f % 128 == 0 and o_shard % 128 == 0 and t_tokens % tc_size == 0
    kt = in_f // 128
    n_ot = o_shard // 128
    n_tc = t_tokens // tc_size

    nc = bacc.Bacc("TRN2", target_bir_lowering=False, debug=False)

    xt_d = nc.dram_tensor("xt", [in_f, t_tokens], DT.float16, kind="ExternalInput")
    wt_d = nc.dram_tensor("wt", [in_f, o_shard], DT.float8e3, kind="ExternalInput")
    scl_d = nc.dram_tensor("scl", [128, n_ot], DT.float32, kind="ExternalInput")
    bsv_d = nc.dram_tensor("bsv", [128, n_ot], DT.float32, kind="ExternalInput")
    yt_d = nc.dram_tensor("yt", [o_shard, t_tokens], DT.float16, kind="ExternalOutput")

    with tile.TileContext(nc) as tc:
        with (
            tc.tile_pool(name="const", bufs=1) as cpool,
            tc.tile_pool(name="wt", bufs=kt) as wtp,
            tc.tile_pool(name="xtp", bufs=kt + x_extra_bufs) as xtp,
            tc.tile_pool(name="outp", bufs=out_bufs) as outp,
            tc.tile_pool(name="ps", bufs=8, space=bass.MemorySpace.PSUM) as psp,
        ):
            # PE warmup: a few dummy matmuls on a zeroed tile start the HAM
            # activity window while the first DMAs land, so the real matmul
            # stream reaches the warm 2.4 GHz clock sooner.
            if warmup_mms:
                warm_t = cpool.tile([128, 128], DT.float16, tag="warm")
                nc.gpsimd.memset(warm_t[:], 0.0)
                warm_ps = psp.tile([128, 128], DT.float32, tag="ps", name="warm")
                for _ in range(warmup_mms):
                    nc.tensor.matmul(
                        warm_ps[:], warm_t[:], warm_t[:], start=True, stop=True
                    )

            # Tiny constant loads go first on each HW DGE queue: they warm
            # the DMA path so the first W / x transfers run at full rate.
            bsv_t = cpool.tile([128, n_ot], DT.float32, tag="bsv")
            nc.sync.dma_start(bsv_t[:], bsv_d[:])
            scl_t = cpool.tile([128, n_ot], DT.float32, tag="scl")
            nc.scalar.dma_start(scl_t[:], scl_d[:])

            # W k-tile loads on the sync queue; chunk-0 x goes on the scalar
            # queue so the two transfer concurrently during the ramp.
            wts = []
            w0_split = min(3, n_ot) * 128
            for k in range(kt):
                w = wtp.tile([128, o_shard], DT.float8e3, tag="wt")
                if k == 0 and w0_split < o_shard:
                    # First k-tile split across two queues: the ramp's first
                    # matmuls need only the leading o-columns, which land on
                    # the otherwise-idle gpsimd queue right after the memset.
                    nc.gpsimd.dma_start(w[:, 0:w0_split], wt_d[0:128, 0:w0_split])
                    nc.sync.dma_start(
                        w[:, w0_split:o_shard], wt_d[0:128, w0_split:o_shard]
                    )
                else:
                    nc.sync.dma_start(w[:], wt_d[k * 128 : (k + 1) * 128, :])
                wts.append(w)

            def load_chunk(tci):
                xts = []
                for k in range(kt):
                    if tci == 0:
                        eng = nc.gpsimd if k == 0 else nc.scalar
                    else:
                        eng = nc.sync
                    xt_t = xtp.tile([128, tc_size], DT.float16, tag="xt")
                    eng.dma_start(
                        xt_t[:],
                        xt_d[
                            k * 128 : (k + 1) * 128,
                            tci * tc_size : (tci + 1) * tc_size,
                        ],
                    )
                    xts.append(xt_t)
                return xts

            def drain_store(ps, ot, tci, t0=0, tn=None):
                tn = tc_size if tn is None else tn
                out_t = outp.tile([128, tn], DT.float16, tag="out")
                nc.vector.tensor_scalar(
                    out_t[:],
                    ps[:],
                    scl_t[:, ot : ot + 1],
                    bsv_t[:, ot : ot + 1],
                    op0=ALU.mult,
                    op1=ALU.add,
                )
                nc.scalar.dma_start(
                    yt_d[
                        ot * 128 : (ot + 1) * 128,
                        tci * tc_size + t0 : tci * tc_size + t0 + tn,
                    ],
                    out_t[:],
                )

            def mm_group(xts, ot, tci, t0=0, tn=None):
                tn = tc_size if tn is None else tn
                ps = psp.tile([128, tn], DT.float32, tag="ps")
                for k in range(kt):
                    nc.tensor.matmul(
                        ps[:],
                        wts[k][:, ot * 128 : (ot + 1) * 128],
                        xts[k][:, t0 : t0 + tn],
                        start=(k == 0),
                        stop=(k == kt - 1),
                    )
                drain_store(ps, ot, tci, t0, tn)

            for tci in range(n_tc):
                xts = load_chunk(tci)
                if tci == 0 and ramp_banks:
                    # k-outer across `ramp_banks` PSUM banks: the PE consumes
                    # each (W, x) k-tile pair the moment the DMA delivers it.
                    ra = list(range(min(ramp_banks, n_ot)))
                    pss = {
                        ot: psp.tile([128, tc_size], DT.float32, tag="ps", name="ps")
                        for ot in ra
                    }
                    for k in range(kt):
                        for ot in ra:
                            nc.tensor.matmul(
                                pss[ot][:],
                                wts[k][:, ot * 128 : (ot + 1) * 128],
                                xts[k][:],
                                start=(k == 0),
                                stop=(k == kt - 1),
                            )
                    for ot in ra:
                        drain_store(pss[ot], ot, tci)
                    rest = range(len(ra), n_ot)
                else:
                    rest = range(n_ot)
                for ot in rest:
                    if tci == n_tc - 1 and ot == n_ot - 1 and tc_size >= 512:
                        h = tc_size // 2
                        q = tc_size // 4
                        mm_group(xts, ot, tci, 0, h)
                        mm_group(xts, ot, tci, h, q)
                        mm_group(xts, ot, tci, h + q, q)
                    else:
                        mm_group(xts, ot, tci)

    nc.compile()
    return nc


def quant_levels(levels):
    """Quantize the 4 levels to fp8 e3m4 with the best global pre-scale s
    (folded back into the per-output drain scale)."""
    lv = np.asarray(levels, dtype=np.float64)
    best = (np.inf, 1.0)
    for sc in np.geomspace(0.25, 4.0, 257):
        q = np.asarray(lv / sc, dtype=np.float32).astype(
            ml_dtypes.float8_e3m4).astype(np.float64) * sc
        r = float(np.sqrt(np.mean((q - lv) ** 2)))
        if r < best[0]:
            best = (r, float(sc))
    sc = best[1]
    lut = np.asarray(lv / sc, dtype=np.float32).astype(ml_dtypes.float8_e3m4)
    return lut, sc


def make_in_maps(x, levels, weight_indices, weight_scales, bias):
    """Host-side shard + layout prep: one input map per core."""
    t_tokens = x.shape[0] * x.shape[1]
    in_f = x.shape[2]
    o_shard = weight_indices.shape[0] // NCORES
    n_ot = o_shard // 128

    x2 = np.asarray(x, dtype=np.float32).reshape(t_tokens, in_f)
    xt = np.ascontiguousarray(x2.T).astype(np.float16)

    # Host dequant: W^T[k, o] = e3m4(levels/s)[idx^T[k, o]]; s and the
    # per-output scale are folded into the drain.
    lut, lv_scale = quant_levels(levels)
    wt_full = lut[np.asarray(weight_indices).T]  # [IN_F, OUT_F] fp8e3

    in_maps = []
    for c in range(NCORES):
        o0, o1 = c * o_shard, (c + 1) * o_shard
        wt = np.ascontiguousarray(wt_full[:, o0:o1])
        scl = np.ascontiguousarray(
            (np.asarray(weight_scales[o0:o1], dtype=np.float64) * lv_scale)
            .astype(np.float32).reshape(n_ot, 128).T
        )
        bsv = np.ascontiguousarray(
            np.asarray(bias[o0:o1], dtype=np.float32).reshape(n_ot, 128).T
        )
        in_maps.append({"xt": xt, "wt": wt, "scl": scl, "bsv": bsv})
    return in_maps


_PROGRAM_CACHE: dict = {}


def _get_program():
    if "p" not in _PROGRAM_CACHE:
        _PROGRAM_CACHE["p"] = build_program(
            in_f=IN_F, t_tokens=T_TOKENS, o_shard=O_SHARD
        )
    return _PROGRAM_CACHE["p"]


def run_on_cores(x, levels, weight_indices, weight_scales, bias, *, trace=False):
    nc = _get_program()
    in_maps = make_in_maps(x, levels, weight_indices, weight_scales, bias)
    res = run_bass_kernel_spmd(
        nc, in_maps, core_ids=list(range(NCORES)), trace=trace
    )
    yt = np.concatenate([res.results[c]["yt"] for c in range(NCORES)], axis=0)
    y = np.ascontiguousarray(yt.T).astype(np.float32).reshape(B, S, OUT_F)
    return y, res


def kernel(x, levels, weight_indices, weight_scales, bias):
    y, _ = run_on_cores(x, levels, weight_indices, weight_scales, bias)
    return y
